# revision 18
# baseline (speedup 1.0000x reference)
"""TRN2 Bass kernel for nn_CSWinB (CSWin attention block), 8-core SPMD.

Sharding: core c = (batch b=c//2, branch br=c%2). Host sums the two
half-contraction partials per batch.

v2 redesign (366us baseline -> target ~180us):
- LN stats via Gram trick: ssq = x^T (W W^T) x, mu = mvec^T x -- the full
  512-channel embed is never computed. Embed+LN-scale+qkv fold into single
  256x256 weights W~ = (W - mvec 1^T) diag(g1) Wqkv applied to xs = x*a.
- fp8e4 DoubleRow (0.5 cyc/row) for: qkv GEMMs, scores (q/k produced in a
  DR layout via host-permuted weight columns: head h at partition band
  32*(h%4), d-halves split along free dim; two head-groups), PV (ones-column
  softmax-sum trick), all-9-tap LePE (diagonal-pair DR matmuls on a
  row+col zero-padded window image), proj.
- bf16 for precision-critical paths: Gram stats, y0 residual GEMM, v GEMM
  and vout GEMM (fp8 GEMM noise does not sqrt(N)-average on zero-mean dots).
- ACT runs exp only (+ln/exp rsqrt, square): all funcs live in the
  natural_log_exp_and_others table => zero ACT table reloads.
- k softmax bias dropped (cancels in softmax); q/v biases folded in as
  K=1 outer-product matmuls; all scales power-of-2, folded into weights so
  every PSUM->SBUF copy is a plain cast (Pool engine).
- y0w GEMM accumulates into the proj PSUM group (wy pre-scaled 2^10), so
  out = (psum)*2^-10 + b2 in one DVE tensor_scalar.
"""
import sys
sys.path.insert(0, '/opt/trn_rl_repo')
from contextlib import ExitStack

import numpy as np
import ml_dtypes

import concourse.bass as bass
import concourse.tile as tile
import concourse.mybir as mybir
from concourse import bacc
from concourse.bass_utils import run_bass_kernel_spmd

# Force the activation-table selector to use natural_log_exp_and_others for
# Exp/Ln/Square/Copy/Identity (it greedily picks the first table containing
# each func, thrashing 1.3us ACT_TABLE_LOADs between exp- and ln-only
# tables). Keys/order preserved so act_func_set ids stay valid.
_orig_get_tables = None


def _patched_tables(arch):
    import concourse.hw_specs as hs
    tabs = dict(_orig_get_tables(arch))
    keep = {'exp', 'ln', 'square', 'copy', 'identity'}
    out = {}
    for name, funcs in tabs.items():
        if name == 'natural_log_exp_and_others':
            out[name] = funcs
        else:
            out[name] = {f for f in funcs
                         if f.name.lower() not in keep}
    return out


def _install_table_patch():
    global _orig_get_tables
    if _orig_get_tables is None:
        _orig_get_tables = bacc.get_activation_tables
        bacc.get_activation_tables = _patched_tables

B, DIM = 4, 256
L = 4096
C2, CB, NH, HD = 512, 256, 8, 32
SCALE = HD ** -0.5
EPS = 1e-5
NWIN, WIN = 8, 512
R, C = 64, 8            # unified window image
CP = C + 2              # zero-padded columns
BLK = (R + 2) * CP + 4  # row+col padded image + spare for DR pair reads
LW = 640                # lepe psum width: 64 rows x 10 padded cols

EQ = 6                  # q/k weight scale 2^EQ (q8 = q*2^EQ)
SV = 2                  # v8 = v*2^SV
EL = 6                  # ldiag = w9*2^EL
EW2 = 8                 # w28 = w2*2^EW2
SO = 2                  # otf = otf_true*2^SO  (== SV so PV ones cols = 1.0)
EMU = 10                # mv8 = mvec*2^EMU

f32 = mybir.dt.float32
f32r = mybir.dt.float32r
bf16 = mybir.dt.bfloat16
fp8 = mybir.dt.float8e4
DR = mybir.MatmulPerfMode.DoubleRow
AF = mybir.ActivationFunctionType
ALU = mybir.AluOpType

# 9 lepe taps + 1 zero tap, as 5 DR pairs; tap offset in padded image = 10*dy+dx
TAPS9 = [(dy, dx) for dy in (-1, 0, 1) for dx in (-1, 0, 1)]


def _ap(t, off, pattern):
    return bass.AP(tensor=t.tensor, offset=t.offset + off,
                   ap=[t.ap[0]] + pattern)


def build_nc():
    _install_table_patch()
    nc = bacc.Bacc("TRN2", target_bir_lowering=False, debug=False)
    xb16d = nc.dram_tensor("xb16", [128, 2 * L], bf16, kind="ExternalInput").ap()
    g16d = nc.dram_tensor("g16", [128, 512], bf16, kind="ExternalInput").ap()
    mv16d = nc.dram_tensor("mv16", [128, 2], bf16, kind="ExternalInput").ap()
    wy16d = nc.dram_tensor("wy16", [128, 512], bf16, kind="ExternalInput").ap()
    wv16d = nc.dram_tensor("wv16", [128, 512], bf16, kind="ExternalInput").ap()
    wob16d = nc.dram_tensor("wob16", [128, 512], bf16, kind="ExternalInput").ap()
    wqk8d = nc.dram_tensor("wqk8", [128, 2048], fp8, kind="ExternalInput").ap()
    qbd = nc.dram_tensor("qb", [128, 4], f32, kind="ExternalInput").ap()
    vb16d = nc.dram_tensor("vb16", [1, 256], bf16, kind="ExternalInput").ap()
    ld8d = nc.dram_tensor("ld8", [128, 2560], fp8, kind="ExternalInput").ap()
    w28d = nc.dram_tensor("w28", [128, 512], fp8, kind="ExternalInput").ap()
    b2cd = nc.dram_tensor("b2c", [128, 2], f32, kind="ExternalInput").ap()
    ident16d = nc.dram_tensor("ident16", [128, 128], bf16, kind="ExternalInput").ap()
    outp = nc.dram_tensor("outp", [256, L], f32, kind="ExternalOutput").ap()
    voutp = nc.dram_tensor("voutp", [256, L], f32, kind="ExternalOutput").ap()

    with tile.TileContext(nc) as tc, ExitStack() as ctx:
        const = ctx.enter_context(tc.tile_pool(name="const", bufs=1))
        big = ctx.enter_context(tc.tile_pool(name="big", bufs=1))

        # ---------- constants ----------
        def cload(name, dram, shape, dt):
            t = const.tile(shape, dt, tag=name)
            nc.gpsimd.dma_start(t[:], dram[:])
            return t

        g16_sb = cload("g16", g16d, [128, 512], bf16)
        mv16_sb = cload("mv16", mv16d, [128, 2], bf16)
        wy16_sb = cload("wy16", wy16d, [128, 512], bf16)
        wv16_sb = cload("wv16", wv16d, [128, 512], bf16)
        wob16_sb = cload("wob16", wob16d, [128, 512], bf16)
        wqk8_sb = cload("wqk8", wqk8d, [128, 2048], fp8)
        qb_sb = cload("qb", qbd, [128, 4], f32)
        vb16_sb = cload("vb16", vb16d, [1, 256], bf16)
        ld8_sb = cload("ld8", ld8d, [128, 2560], fp8)
        w28_sb = cload("w28", w28d, [128, 512], fp8)
        b2c_sb = cload("b2c", b2cd, [128, 2], f32)
        ident16_sb = cload("ident16", ident16d, [128, 128], bf16)

        ones16_sb = const.tile([128, 1], bf16, tag="ones16")
        nc.gpsimd.memset(ones16_sb[:], 1.0)
        ones8r_sb = const.tile([1, 1024], fp8, tag="ones8r")
        nc.gpsimd.memset(ones8r_sb[:], 2.0 ** -6)
        ones16r_sb = const.tile([1, 512], bf16, tag="ones16r")
        nc.gpsimd.memset(ones16r_sb[:], 1.0)

        # ---------- persistent activations (manual double-buffer) ----------
        va_sb = [big.tile([128, 2048], fp8, name=f"vasb{i}") for i in range(2)]
        for i in range(2):
            # ones columns at 512*jt + 64h + 0..32; value 2^(SV-SO) = 1.0
            dst = _ap(va_sb[i], 0, [[512, 4], [64, 8], [1, 32]])
            nc.gpsimd.memset(dst, 1.0)
        vpw_sb = [[big.tile([128, BLK], fp8, name=f"vpw{ct}{i}")
                   for i in range(2)] for ct in range(2)]
        for ct in range(2):
            for i in range(2):
                t = vpw_sb[ct][i]
                nc.gpsimd.memset(_ap(t, 0, [[1, CP]]), 0.0)            # row -1
                nc.gpsimd.memset(_ap(t, (R + 1) * CP, [[1, BLK - (R + 1) * CP]]), 0.0)
                nc.gpsimd.memset(_ap(t, CP, [[CP, R], [CP - 1, 2]]), 0.0)

        pools = {}
        for nm, bufs, space in [
                ("xb16p", 2, None), ("p16p", 2, None),
                ("smp", 2, None), ("xs8p", 2, None), ("xs16p", 2, None),
                ("qkp", 2, None), ("v8p", 2, None), ("v16p", 2, None),
                ("otfp", 2, None), ("outfp", 2, None), ("recp", 4, None),
                ("abp", 2, None), ("exp_", 4, None),
                ("pE", 2, "PSUM"), ("pST", 2, "PSUM"), ("pM", 2, "PSUM")]:
            kw = dict(name=nm, bufs=bufs)
            if space:
                kw["space"] = space
            pools[nm] = ctx.enter_context(tc.tile_pool(**kw))
        xb16p, p16p, smp = (pools[k] for k in
                              ("xb16p", "p16p", "smp"))
        xs8p, xs16p, qkp, v8p, v16p = (pools[k] for k in
                                       ("xs8p", "xs16p", "qkp", "v8p", "v16p"))
        otfP, outfP, recP, abP, exP = (pools[k] for k in
                                       ("otfp", "outfp", "recp", "abp", "exp_"))
        pE, pST, pM = (pools[k] for k in ("pE", "pST", "pM"))

        def emit_A1(w):
            sl = slice(w * 512, (w + 1) * 512)
            s = {"sl": sl, "w": w}
            xb16_t = xb16p.tile([128, 1024], bf16, tag="xb16")
            nc.sync.dma_start(xb16_t[:], _ap(xb16d, w * 512, [[L, 2], [1, 512]]))
            gx_ps = []
            for ot in range(2):
                ps = pE.tile([128, 512], f32, tag="pe")
                for kt in range(2):
                    nc.tensor.matmul(ps[:],
                                     g16_sb[:, kt * 256 + ot * 128:kt * 256 + (ot + 1) * 128],
                                     xb16_t[:, kt * 512:(kt + 1) * 512],
                                     start=(kt == 0), stop=(kt == 1))
                gx_ps.append(ps)
            p16_t = p16p.tile([128, 1024], bf16, tag="p16")
            for ot in range(2):
                nc.vector.tensor_mul(p16_t[:, ot * 512:(ot + 1) * 512],
                                     gx_ps[ot][:],
                                     xb16_t[:, ot * 512:(ot + 1) * 512])
            mu_ps = pE.tile([1, 512], f32, tag="pe")
            for kt in range(2):
                nc.tensor.matmul(mu_ps[:], mv16_sb[:, kt:kt + 1],
                                 xb16_t[:, kt * 512:(kt + 1) * 512],
                                 start=(kt == 0), stop=(kt == 1))
            s.update(xb16_t=xb16_t, p16=p16_t, mu_ps=mu_ps)
            return s

        def emit_stats(s):
            ssq_ps = pE.tile([1, 512], f32, tag="pe")
            for kt in range(2):
                nc.tensor.matmul(ssq_ps[:], ones16_sb[:],
                                 s["p16"][:, kt * 512:(kt + 1) * 512],
                                 start=(kt == 0), stop=(kt == 1))
            mu2 = smp.tile([1, 512], f32, tag="mu2")
            nc.scalar.activation(mu2[:], s["mu_ps"][:], AF.Square)
            var0 = smp.tile([1, 512], f32, tag="var0")
            nc.vector.tensor_scalar(var0[:], ssq_ps[:], 1.0 / C2, EPS,
                                    op0=ALU.mult, op1=ALU.add)
            var = smp.tile([1, 512], f32, tag="var")
            nc.vector.scalar_tensor_tensor(var[:], mu2[:], -1.0, var0[:],
                                           op0=ALU.mult, op1=ALU.add)
            lnv = smp.tile([1, 512], f32, tag="lnv")
            nc.scalar.activation(lnv[:], var[:], AF.Ln)
            a16 = smp.tile([1, 512], bf16, tag="a16")
            nc.scalar.activation(a16[:], lnv[:], AF.Exp, scale=-0.5)
            a_b = abP.tile([128, 512], bf16, tag="ab")
            nc.gpsimd.partition_broadcast(a_b[:], a16[:])
            s["a_b"] = a_b

        def emit_xs(s):
            xs8_t = xs8p.tile([128, 1024], fp8, tag="xs8")
            xs16_t = xs16p.tile([128, 1024], bf16, tag="xs16")
            for ct in range(2):
                cs = slice(ct * 512, (ct + 1) * 512)
                nc.vector.tensor_mul(xs16_t[:, cs], s["xb16_t"][:, cs],
                                     s["a_b"][:])
            for ct in range(2):
                cs = slice(ct * 512, (ct + 1) * 512)
                nc.vector.tensor_mul(xs8_t[:, cs], s["xb16_t"][:, cs],
                                     s["a_b"][:])
            s.update(xs8_t=xs8_t, xs16_t=xs16_t)

        def emit_qk(s):
            # 8 blocks: q(g,dhi) x4 then k(g,dhi) x4; q gets the fp8 bias matmul
            qdr = [qkp.tile([128, 1024], fp8, tag=f"q{g}", name=f"qdr{g}")
                   for g in range(2)]
            kdr = [qkp.tile([128, 1024], fp8, tag=f"k{g}", name=f"kdr{g}")
                   for g in range(2)]
            rhs = _ap(s["xs8_t"], 0, [[512, 2], [1, 512]])
            for m in range(8):
                is_q = m < 4
                g, dhi = (m % 4) // 2, m % 2
                ps = pE.tile([128, 512], f32, tag="pe")
                nc.tensor.matmul(ps[:],
                                 _ap(wqk8_sb, m * 256, [[128, 2], [1, 128]]),
                                 rhs, perf_mode=DR, start=True, stop=True)
                dst = (qdr if is_q else kdr)[g][:, dhi * 512:(dhi + 1) * 512]
                if is_q:
                    nc.scalar.activation(dst, ps[:], AF.Identity,
                                         bias=qb_sb[:, m:m + 1])
                else:
                    nc.scalar.copy(dst, ps[:])
            s.update(qdr=qdr, kdr=kdr)

        def emit_v(s, w):
            v8_t = v8p.tile([128, 1024], fp8, tag="v8")
            v16_t = v16p.tile([128, 1024], bf16, tag="v16")
            for ot in range(2):
                ps = pE.tile([128, 512], f32, tag="pe")
                for kt in range(2):
                    nc.tensor.matmul(ps[:],
                                     wv16_sb[:, kt * 256 + ot * 128:kt * 256 + (ot + 1) * 128],
                                     s["xs16_t"][:, kt * 512:(kt + 1) * 512],
                                     start=(kt == 0), stop=False)
                nc.tensor.matmul(ps[:], vb16_sb[:, ot * 128:(ot + 1) * 128],
                                 ones16r_sb[:], start=False, stop=True)
                cs = slice(ot * 512, (ot + 1) * 512)
                nc.vector.tensor_copy(v8_t[:, cs], ps[:])
                nc.vector.tensor_copy(v16_t[:, cs], ps[:])
            for ot2 in range(2):
                ps2 = pE.tile([128, 512], f32, tag="pe")
                for kt in range(2):
                    nc.tensor.matmul(ps2[:],
                                     wob16_sb[:, kt * 256 + ot2 * 128:kt * 256 + (ot2 + 1) * 128],
                                     v16_t[:, kt * 512:(kt + 1) * 512],
                                     start=(kt == 0), stop=(kt == 1))
                voe = v16p.tile([128, 512], f32, tag=f"voe{ot2}")
                nc.vector.tensor_copy(voe[:], ps2[:])
                nc.sync.dma_start(voutp[ot2 * 128:(ot2 + 1) * 128, s["sl"]],
                                  voe[:])
            va = va_sb[w % 2]
            for ct in range(2):
                trp = pM.tile([128, 512], bf16, tag="pm")
                for jt in range(4):
                    nc.tensor.transpose(
                        trp[:, jt * 128:(jt + 1) * 128],
                        v16_t[:, ct * 512 + jt * 128:ct * 512 + (jt + 1) * 128],
                        ident16_sb[:])
                dst = _ap(va, 256 * ct + 32, [[512, 4], [64, 4], [1, 32]])
                nc.vector.tensor_copy(
                    dst, trp[:].rearrange("p (a b c) -> p a b c", a=4, b=4))
            vpw = [vpw_sb[ct][w % 2] for ct in range(2)]
            for ct in range(2):
                dst = _ap(vpw[ct], CP + 1, [[CP, R], [1, C]])
                vsrc = v8_t[:, ct * 512:(ct + 1) * 512].rearrange(
                    "p (r c) -> p r c", r=R)
                nc.gpsimd.tensor_copy(dst, vsrc)
            s.update(v8_t=v8_t, va=va, vpw=vpw)
            otf_t = otfP.tile([128, 1024], fp8, tag="otf")
            s["otf_t"] = otf_t

        def emit_heads(s, h0, h1):
            qdr, kdr, va, otf_t = s["qdr"], s["kdr"], s["va"], s["otf_t"]
            for hp in range(h0, h1, 2):
                pair = (hp, hp + 1)
                exs = {}
                for h in pair:
                    exs[h] = exP.tile([128, 2048], fp8, tag="ex",
                                      name=f"ex{h % 2}")
                for half in range(2):
                    sts = {}
                    for h in pair:
                        sts[h] = pST.tile([128, 1024], f32, tag="st",
                                          name=f"st{h % 2}")
                    for j2 in range(2):
                        jt = half * 2 + j2
                        for h in pair:
                            g, bnd = h // 4, h % 4
                            kb = kdr[g][32 * bnd:32 * bnd + 16, :]
                            lhs = bass.AP(tensor=kb.tensor,
                                          offset=kb.offset + jt * 128,
                                          ap=[kb.ap[0], [512, 2], [1, 128]])
                            qb = qdr[g][32 * bnd:32 * bnd + 16, :]
                            rhs = bass.AP(tensor=qb.tensor,
                                          offset=qb.offset,
                                          ap=[qb.ap[0], [512, 2], [1, 512]])
                            nc.tensor.matmul(
                                sts[h][:, j2 * 512:(j2 + 1) * 512],
                                lhs, rhs, perf_mode=DR,
                                start=True, stop=True,
                                tile_position=(32 * bnd, 0))
                    for h in pair:
                        nc.scalar.activation(
                            exs[h][:, half * 1024:(half + 1) * 1024],
                            sts[h][:], AF.Exp, scale=2.0 ** (-2 * EQ))
                for h in pair:
                    cth, hh = h // 4, h % 4
                    pv = pM.tile([64, 512], f32, tag="pm")
                    for half in range(2):
                        lhs = _ap(va, 1024 * half + 64 * h, [[512, 2], [1, 64]])
                        rhs = _ap(exs[h], 1024 * half, [[512, 2], [1, 512]])
                        nc.tensor.matmul(pv[:], lhs, rhs, perf_mode=DR,
                                         start=(half == 0), stop=(half == 1))
                    rec = recP.tile([32, 512], f32, tag="rec")
                    nc.vector.reciprocal_approx_fast(rec[:], pv[0:32, :])
                    nc.vector.tensor_mul(
                        otf_t[32 * hh:32 * hh + 32, cth * 512:(cth + 1) * 512],
                        pv[32:64, :], rec[:])

        def emit_tail(s):
            otf_t = s["otf_t"]
            # tap pairs: (t0, zero), (t1,t2), (t3,t4), (t5,t6), (t7,t8);
            # contiguous 640-wide reads over the padded image (cols 8,9 junk)
            PAIRS = [(0, None), (1, 2), (3, 4), (5, 6), (7, 8)]
            for ct in range(2):
                lp = pST.tile([128, LW], f32, tag="st", name="lp")
                for c0, cw in ((0, 512), (512, LW - 512)):
                    for pr, (ta_i, tb_i) in enumerate(PAIRS):
                        ta = TAPS9[ta_i]
                        off_a = CP * ta[0] + ta[1]
                        if tb_i is not None:
                            tb = TAPS9[tb_i]
                            delta = CP * tb[0] + tb[1] - off_a
                        else:
                            delta = 1
                        rhs = _ap(s["vpw"][ct], CP + 1 + off_a + c0,
                                  [[delta, 2], [1, cw]])
                        nc.tensor.matmul(
                            _ap(lp, c0, [[1, cw]]),
                            _ap(ld8_sb, (ct * 5 + pr) * 256,
                                [[128, 2], [1, 128]]),
                            rhs, perf_mode=DR,
                            start=(pr == 0), stop=(pr == 4),
                            skip_group_check=(pr not in (0, 4)))
                cs = slice(ct * 512, (ct + 1) * 512)
                lp_v = _ap(lp, 0, [[CP, R], [1, C]])
                otf_v = otf_t[:, cs].rearrange("p (r c) -> p r c", r=R)
                nc.vector.scalar_tensor_tensor(
                    otf_v, lp_v, 2.0 ** (SO - SV - EL), otf_v,
                    op0=ALU.mult, op1=ALU.add)
            for ot in range(2):
                pp = pM.tile([128, 512], f32, tag="pm")
                for kt in range(2):
                    nc.tensor.matmul(pp[:],
                                     wy16_sb[:, kt * 256 + ot * 128:kt * 256 + (ot + 1) * 128],
                                     s["xb16_t"][:, kt * 512:(kt + 1) * 512],
                                     start=(kt == 0), stop=False)
                nc.tensor.matmul(pp[:],
                                 _ap(w28_sb, ot * 256, [[128, 2], [1, 128]]),
                                 _ap(otf_t, 0, [[512, 2], [1, 512]]),
                                 perf_mode=DR, start=False, stop=True)
                outf = outfP.tile([128, 512], f32, tag="outf")
                nc.vector.tensor_scalar(outf[:], pp[:], 2.0 ** -(EW2 + SO),
                                        b2c_sb[:, ot:ot + 1],
                                        op0=ALU.mult, op1=ALU.add)
                nc.sync.dma_start(outp[ot * 128:(ot + 1) * 128, s["sl"]],
                                  outf[:])

        # prologue
        cur = emit_A1(0)
        emit_stats(cur)
        emit_xs(cur)
        emit_qk(cur)
        emit_v(cur, 0)
        for w in range(8):
            nxt = emit_A1(w + 1) if w + 1 < 8 else None
            emit_heads(cur, 0, 2)
            if nxt:
                emit_stats(nxt)
            emit_heads(cur, 2, 4)
            if nxt:
                emit_xs(nxt)
            emit_heads(cur, 4, 6)
            if nxt:
                emit_v(nxt, w + 1)
            emit_heads(cur, 6, 8)
            if nxt:
                emit_qk(nxt)
            emit_tail(cur)
            cur = nxt

    nc.compile()
    return nc


# ---------------------------------------------------------------------------
# host side
# ---------------------------------------------------------------------------

def _perms():
    p0 = np.arange(L).reshape(64, 8, 8).transpose(1, 0, 2).ravel()
    p1 = np.arange(L).reshape(8, 8, 64).transpose(0, 2, 1).ravel()
    return p0, p1


F8 = ml_dtypes.float8_e4m3fn
BF = ml_dtypes.bfloat16


def _qk_perm():
    """Channel permutation for q/k dr layout.

    Block m = (is_k, g, dhi); psum partition p: band h'=p//32, r=p%32;
    r<16 -> source channel (4g+h')*32 + dhi*16 + r, else junk (zero col).
    Returns [8, 128] channel index or -1."""
    idx = np.full((8, 128), -1, np.int64)
    for m in range(8):
        g, dhi = (m % 4) // 2, m % 2
        for p in range(128):
            hp, r = p // 32, p % 32
            if r < 16:
                idx[m, p] = (4 * g + hp) * 32 + dhi * 16 + r
    return idx


def _host_prep(inputs):
    x = np.asarray(inputs['x'], np.float32)
    w_embed = np.asarray(inputs['w_embed'], np.float32)
    g1 = np.asarray(inputs['g1'], np.float32)
    b1 = np.asarray(inputs['b1'], np.float32)
    w_qkv = np.asarray(inputs['w_qkv'], np.float32)
    w_proj = np.asarray(inputs['w_proj'], np.float32)
    b_proj = np.asarray(inputs['b_proj'], np.float32)
    w_out = np.asarray(inputs['w_out'], np.float32)
    conv_w = [np.asarray(inputs['conv_w0'], np.float32),
              np.asarray(inputs['conv_w1'], np.float32)]
    conv_b = [np.asarray(inputs['conv_b0'], np.float32),
              np.asarray(inputs['conv_b1'], np.float32)]
    perms = _perms()
    qkidx = _qk_perm()

    mvec = w_embed.mean(axis=1)
    G = w_embed @ w_embed.T
    Wm = w_embed - mvec[:, None]
    ident = np.eye(128, dtype=np.float32).astype(BF)

    def pack_kt_ot(M):
        # [256 in, 256 out] -> [128, 512]: [k_lo, kt*256 + m]
        out = np.zeros((128, 512), np.float32)
        for kt in range(2):
            out[:, kt * 256:(kt + 1) * 256] = M[kt * 128:(kt + 1) * 128, :]
        return out

    in_maps = []
    for c in range(8):
        b, br = c // 2, c % 2
        pm = perms[br]
        o = br * 256
        xp = np.ascontiguousarray(x[b].reshape(256, L)[:, pm])
        xb8 = np.concatenate([xp[:128], xp[128:]], axis=1)  # [128, 2L]
        gWq = g1[:, None] * w_qkv[:, o:o + 256] * SCALE
        gWk = g1[:, None] * w_qkv[:, 512 + o:512 + o + 256]
        gWv = g1[:, None] * w_qkv[:, 1024 + o:1024 + o + 256]
        Wq_ = Wm @ gWq
        Wk_ = Wm @ gWk
        Wv_ = Wm @ gWv
        bq = (b1 @ w_qkv[:, o:o + 256]) * SCALE
        bv = b1 @ w_qkv[:, 1024 + o:1024 + o + 256]
        woutb = w_out[o:o + 256]
        wy = w_embed[:, o:o + 256] @ woutb
        w2 = w_proj[o:o + 256] @ w_out
        b2 = ((b_proj @ w_out) if br == 0 else np.zeros(256, np.float32)) \
            + conv_b[br] @ w2
        w9 = conv_w[br].reshape(256, 3, 3)
        if br == 1:
            w9 = w9.transpose(0, 2, 1)
        w9 = np.ascontiguousarray(w9.reshape(256, 9))

        # wqk8: 8 blocks of [128, 2 planes x 128 cols]
        wqk8 = np.zeros((128, 2048), np.float32)
        qb = np.zeros((128, 4), np.float32)
        for m in range(8):
            W = Wq_ if m < 4 else Wk_
            for kt in range(2):
                blk = wqk8[:, m * 256 + kt * 128:m * 256 + (kt + 1) * 128]
                for p in range(128):
                    ch = qkidx[m, p]
                    if ch >= 0:
                        blk[:, p] = W[kt * 128:(kt + 1) * 128, ch] * 2.0 ** EQ
            if m < 4:  # q bias column for the ACT Identity copy
                for p in range(128):
                    ch = qkidx[m, p]
                    if ch >= 0:
                        qb[p, m] = bq[ch] * 2.0 ** EQ

        # ldiag8: (ct, pair) blocks of [128, 2 planes x 128], diag values
        ld8 = np.zeros((128, 2560), np.float32)
        LPAIRS = [(0, None), (1, 2), (3, 4), (5, 6), (7, 8)]
        for ct in range(2):
            for pr in range(5):
                for j, t9 in enumerate(LPAIRS[pr]):
                    if t9 is None:
                        continue
                    blk = ld8[:, (ct * 5 + pr) * 256 + j * 128:
                              (ct * 5 + pr) * 256 + (j + 1) * 128]
                    blk[np.arange(128), np.arange(128)] = \
                        w9[ct * 128:(ct + 1) * 128, t9] * 2.0 ** EL

        w28 = np.zeros((128, 512), np.float32)
        for ot in range(2):
            for kt in range(2):
                w28[:, ot * 256 + kt * 128:ot * 256 + (kt + 1) * 128] = \
                    w2[kt * 128:(kt + 1) * 128,
                       ot * 128:(ot + 1) * 128] * 2.0 ** EW2

        in_maps.append({
            "xb16": xb8.astype(BF),
            "g16": pack_kt_ot(G).astype(BF),
            "mv16": mvec.reshape(2, 128).T.astype(BF),
            "wy16": (pack_kt_ot(wy) * 2.0 ** (EW2 + SO)).astype(BF),
            "wv16": (pack_kt_ot(Wv_) * 2.0 ** SV).astype(BF),
            "wob16": (pack_kt_ot(woutb) * 2.0 ** -SV).astype(BF),
            "wqk8": wqk8.astype(F8),
            "qb": qb,
            "vb16": (bv.reshape(1, 256) * 2.0 ** SV).astype(BF),
            "ld8": ld8.astype(F8),
            "w28": w28.astype(F8),
            "b2c": np.ascontiguousarray(b2.reshape(2, 128).T.astype(np.float32)),
            "ident16": ident,
        })
    return in_maps


def _gather(results):
    perms = _perms()
    out = np.zeros((B, 256, L), np.float32)
    vout = np.zeros((B, 256, L), np.float32)
    for c in range(8):
        b, br = c // 2, c % 2
        pm = perms[br]
        tmp = np.zeros((256, L), np.float32)
        tmp[:, pm] = results[c]["outp"]
        out[b] += tmp
        tmp = np.zeros((256, L), np.float32)
        tmp[:, pm] = results[c]["voutp"]
        vout[b] += tmp
    return (out.reshape(B, 256, 64, 64), vout.reshape(B, 256, 64, 64))


_CACHE = {}


def get_nc():
    if "nc" not in _CACHE:
        _CACHE["nc"] = build_nc()
    return _CACHE["nc"]


def kernel(**inputs):
    nc = get_nc()
    in_maps = _host_prep(inputs)
    res = run_bass_kernel_spmd(nc, in_maps, core_ids=list(range(8)))
    return _gather(res.results)


# revision 19
# speedup vs baseline: 1.2078x; 1.2078x over previous
"""TRN2 Bass kernel for nn_CSWinB (CSWin attention block), 8-core SPMD.

Sharding: core c = (batch b=c//2, branch br=c%2). Host sums the two
half-contraction partials per batch.

v2 redesign (366us baseline -> target ~180us):
- LN stats via Gram trick: ssq = x^T (W W^T) x, mu = mvec^T x -- the full
  512-channel embed is never computed. Embed+LN-scale+qkv fold into single
  256x256 weights W~ = (W - mvec 1^T) diag(g1) Wqkv applied to xs = x*a.
- fp8e4 DoubleRow (0.5 cyc/row) for: qkv GEMMs, scores (q/k produced in a
  DR layout via host-permuted weight columns: head h at partition band
  32*(h%4), d-halves split along free dim; two head-groups), PV (ones-column
  softmax-sum trick), all-9-tap LePE (diagonal-pair DR matmuls on a
  row+col zero-padded window image), proj.
- bf16 for precision-critical paths: Gram stats, y0 residual GEMM, v GEMM
  and vout GEMM (fp8 GEMM noise does not sqrt(N)-average on zero-mean dots).
- ACT runs exp only (+ln/exp rsqrt, square): all funcs live in the
  natural_log_exp_and_others table => zero ACT table reloads.
- k softmax bias dropped (cancels in softmax); q/v biases folded in as
  K=1 outer-product matmuls; all scales power-of-2, folded into weights so
  every PSUM->SBUF copy is a plain cast (Pool engine).
- y0w GEMM accumulates into the proj PSUM group (wy pre-scaled 2^10), so
  out = (psum)*2^-10 + b2 in one DVE tensor_scalar.
"""
import sys
sys.path.insert(0, '/opt/trn_rl_repo')
from contextlib import ExitStack

import numpy as np
import ml_dtypes

import concourse.bass as bass
import concourse.tile as tile
import concourse.mybir as mybir
from concourse import bacc
from concourse.bass_utils import run_bass_kernel_spmd

# Force the activation-table selector to use natural_log_exp_and_others for
# Exp/Ln/Square/Copy/Identity (it greedily picks the first table containing
# each func, thrashing 1.3us ACT_TABLE_LOADs between exp- and ln-only
# tables). Keys/order preserved so act_func_set ids stay valid.
_orig_get_tables = None


def _patched_tables(arch):
    import concourse.hw_specs as hs
    tabs = dict(_orig_get_tables(arch))
    keep = {'exp', 'ln', 'square', 'copy', 'identity'}
    out = {}
    for name, funcs in tabs.items():
        if name == 'natural_log_exp_and_others':
            out[name] = funcs
        else:
            out[name] = {f for f in funcs
                         if f.name.lower() not in keep}
    return out


def _install_table_patch():
    global _orig_get_tables
    if _orig_get_tables is None:
        _orig_get_tables = bacc.get_activation_tables
        bacc.get_activation_tables = _patched_tables

B, DIM = 4, 256
L = 4096
C2, CB, NH, HD = 512, 256, 8, 32
SCALE = HD ** -0.5
EPS = 1e-5
NWIN, WIN = 8, 512
R, C = 64, 8            # unified window image
CP = C + 2              # zero-padded columns
BLK = (R + 2) * CP + 4  # row+col padded image + spare for DR pair reads
LW = 640                # lepe psum width: 64 rows x 10 padded cols

EQ = 6                  # q/k weight scale 2^EQ (q8 = q*2^EQ)
SV = 2                  # v8 = v*2^SV
EL = 6                  # ldiag = w9*2^EL
EW2 = 8                 # w28 = w2*2^EW2
SO = 2                  # otf = otf_true*2^SO  (== SV so PV ones cols = 1.0)
EMU = 10                # mv8 = mvec*2^EMU

f32 = mybir.dt.float32
f32r = mybir.dt.float32r
bf16 = mybir.dt.bfloat16
fp8 = mybir.dt.float8e4
DR = mybir.MatmulPerfMode.DoubleRow
AF = mybir.ActivationFunctionType
ALU = mybir.AluOpType

# 9 lepe taps + 1 zero tap, as 5 DR pairs; tap offset in padded image = 10*dy+dx
TAPS9 = [(dy, dx) for dy in (-1, 0, 1) for dx in (-1, 0, 1)]


def _ap(t, off, pattern):
    return bass.AP(tensor=t.tensor, offset=t.offset + off,
                   ap=[t.ap[0]] + pattern)


def build_nc():
    _install_table_patch()
    nc = bacc.Bacc("TRN2", target_bir_lowering=False, debug=False)
    xb16d = nc.dram_tensor("xb16", [128, 2 * L], bf16, kind="ExternalInput").ap()
    g16d = nc.dram_tensor("g16", [128, 512], bf16, kind="ExternalInput").ap()
    mv16d = nc.dram_tensor("mv16", [128, 2], bf16, kind="ExternalInput").ap()
    wy16d = nc.dram_tensor("wy16", [128, 512], bf16, kind="ExternalInput").ap()
    wv16d = nc.dram_tensor("wv16", [128, 512], bf16, kind="ExternalInput").ap()
    wob16d = nc.dram_tensor("wob16", [128, 512], bf16, kind="ExternalInput").ap()
    wqk8d = nc.dram_tensor("wqk8", [128, 2048], fp8, kind="ExternalInput").ap()
    qbd = nc.dram_tensor("qb", [128, 4], f32, kind="ExternalInput").ap()
    vb16d = nc.dram_tensor("vb16", [1, 256], bf16, kind="ExternalInput").ap()
    ld8d = nc.dram_tensor("ld8", [128, 2560], fp8, kind="ExternalInput").ap()
    w28d = nc.dram_tensor("w28", [128, 512], fp8, kind="ExternalInput").ap()
    b2cd = nc.dram_tensor("b2c", [128, 2], f32, kind="ExternalInput").ap()
    ident16d = nc.dram_tensor("ident16", [128, 128], bf16, kind="ExternalInput").ap()
    outp = nc.dram_tensor("outp", [256, L], f32, kind="ExternalOutput").ap()
    voutp = nc.dram_tensor("voutp", [256, L], f32, kind="ExternalOutput").ap()

    with tile.TileContext(nc) as tc, ExitStack() as ctx:
        const = ctx.enter_context(tc.tile_pool(name="const", bufs=1))
        big = ctx.enter_context(tc.tile_pool(name="big", bufs=1))

        # ---------- constants ----------
        def cload(name, dram, shape, dt):
            t = const.tile(shape, dt, tag=name)
            nc.gpsimd.dma_start(t[:], dram[:])
            return t

        g16_sb = cload("g16", g16d, [128, 512], bf16)
        mv16_sb = cload("mv16", mv16d, [128, 2], bf16)
        wy16_sb = cload("wy16", wy16d, [128, 512], bf16)
        wv16_sb = cload("wv16", wv16d, [128, 512], bf16)
        wob16_sb = cload("wob16", wob16d, [128, 512], bf16)
        wqk8_sb = cload("wqk8", wqk8d, [128, 2048], fp8)
        qb_sb = cload("qb", qbd, [128, 4], f32)
        vb16_sb = cload("vb16", vb16d, [1, 256], bf16)
        ld8_sb = cload("ld8", ld8d, [128, 2560], fp8)
        w28_sb = cload("w28", w28d, [128, 512], fp8)
        b2c_sb = cload("b2c", b2cd, [128, 2], f32)
        ident16_sb = cload("ident16", ident16d, [128, 128], bf16)

        ones16_sb = const.tile([128, 1], bf16, tag="ones16")
        nc.gpsimd.memset(ones16_sb[:], 1.0)
        ones8r_sb = const.tile([1, 1024], fp8, tag="ones8r")
        nc.gpsimd.memset(ones8r_sb[:], 2.0 ** -6)
        ones16r_sb = const.tile([1, 512], bf16, tag="ones16r")
        nc.gpsimd.memset(ones16r_sb[:], 1.0)

        # ---------- persistent activations (manual double-buffer) ----------
        va_sb = [big.tile([128, 2048], fp8, name=f"vasb{i}") for i in range(2)]
        for i in range(2):
            # ones columns at 512*jt + 64h + 0..32; value 2^(SV-SO) = 1.0
            dst = _ap(va_sb[i], 0, [[512, 4], [64, 8], [1, 32]])
            nc.gpsimd.memset(dst, 1.0)
        vpw_sb = [[big.tile([128, BLK], fp8, name=f"vpw{ct}{i}")
                   for i in range(2)] for ct in range(2)]
        for ct in range(2):
            for i in range(2):
                t = vpw_sb[ct][i]
                nc.gpsimd.memset(_ap(t, 0, [[1, CP]]), 0.0)            # row -1
                nc.gpsimd.memset(_ap(t, (R + 1) * CP, [[1, BLK - (R + 1) * CP]]), 0.0)
                nc.gpsimd.memset(_ap(t, CP, [[CP, R], [CP - 1, 2]]), 0.0)

        pools = {}
        for nm, bufs, space in [
                ("xb16p", 2, None), ("p16p", 2, None),
                ("smp", 2, None), ("xs8p", 2, None), ("xs16p", 2, None),
                ("qkp", 2, None), ("v8p", 2, None), ("v16p", 2, None),
                ("otfp", 2, None), ("outfp", 2, None), ("recp", 4, None),
                ("abp", 2, None), ("exp_", 4, None),
                ("pE", 2, "PSUM"), ("pST", 2, "PSUM"), ("pM", 2, "PSUM")]:
            kw = dict(name=nm, bufs=bufs)
            if space:
                kw["space"] = space
            pools[nm] = ctx.enter_context(tc.tile_pool(**kw))
        xb16p, p16p, smp = (pools[k] for k in
                              ("xb16p", "p16p", "smp"))
        xs8p, xs16p, qkp, v8p, v16p = (pools[k] for k in
                                       ("xs8p", "xs16p", "qkp", "v8p", "v16p"))
        otfP, outfP, recP, abP, exP = (pools[k] for k in
                                       ("otfp", "outfp", "recp", "abp", "exp_"))
        pE, pST, pM = (pools[k] for k in ("pE", "pST", "pM"))

        def emit_A1(w):
            sl = slice(w * 512, (w + 1) * 512)
            s = {"sl": sl, "w": w}
            xb16_t = xb16p.tile([128, 1024], bf16, tag="xb16")
            nc.sync.dma_start(xb16_t[:], _ap(xb16d, w * 512, [[L, 2], [1, 512]]))
            gx_ps = []
            for ot in range(2):
                ps = pE.tile([128, 512], f32, tag="pe")
                for kt in range(2):
                    nc.tensor.matmul(ps[:],
                                     g16_sb[:, kt * 256 + ot * 128:kt * 256 + (ot + 1) * 128],
                                     xb16_t[:, kt * 512:(kt + 1) * 512],
                                     start=(kt == 0), stop=(kt == 1))
                gx_ps.append(ps)
            p16_t = p16p.tile([128, 1024], bf16, tag="p16")
            for ot in range(2):
                nc.vector.tensor_mul(p16_t[:, ot * 512:(ot + 1) * 512],
                                     gx_ps[ot][:],
                                     xb16_t[:, ot * 512:(ot + 1) * 512])
            mu_ps = pE.tile([1, 512], f32, tag="pe")
            for kt in range(2):
                nc.tensor.matmul(mu_ps[:], mv16_sb[:, kt:kt + 1],
                                 xb16_t[:, kt * 512:(kt + 1) * 512],
                                 start=(kt == 0), stop=(kt == 1))
            s.update(xb16_t=xb16_t, p16=p16_t, mu_ps=mu_ps)
            return s

        def emit_stats(s):
            ssq_ps = pE.tile([1, 512], f32, tag="pe")
            for kt in range(2):
                nc.tensor.matmul(ssq_ps[:], ones16_sb[:],
                                 s["p16"][:, kt * 512:(kt + 1) * 512],
                                 start=(kt == 0), stop=(kt == 1))
            mu2 = smp.tile([1, 512], f32, tag="mu2")
            nc.scalar.activation(mu2[:], s["mu_ps"][:], AF.Square)
            var0 = smp.tile([1, 512], f32, tag="var0")
            nc.vector.tensor_scalar(var0[:], ssq_ps[:], 1.0 / C2, EPS,
                                    op0=ALU.mult, op1=ALU.add)
            var = smp.tile([1, 512], f32, tag="var")
            nc.vector.scalar_tensor_tensor(var[:], mu2[:], -1.0, var0[:],
                                           op0=ALU.mult, op1=ALU.add)
            lnv = smp.tile([1, 512], f32, tag="lnv")
            nc.scalar.activation(lnv[:], var[:], AF.Ln)
            a16 = smp.tile([1, 512], bf16, tag="a16")
            nc.scalar.activation(a16[:], lnv[:], AF.Exp, scale=-0.5)
            a_b = abP.tile([128, 512], bf16, tag="ab")
            nc.gpsimd.partition_broadcast(a_b[:], a16[:])
            s["a_b"] = a_b

        def emit_xs(s):
            xs8_t = xs8p.tile([128, 1024], fp8, tag="xs8")
            xs16_t = xs16p.tile([128, 1024], bf16, tag="xs16")
            for ct in range(2):
                cs = slice(ct * 512, (ct + 1) * 512)
                nc.vector.tensor_mul(xs16_t[:, cs], s["xb16_t"][:, cs],
                                     s["a_b"][:])
            for ct in range(2):
                cs = slice(ct * 512, (ct + 1) * 512)
                nc.vector.tensor_mul(xs8_t[:, cs], s["xb16_t"][:, cs],
                                     s["a_b"][:])
            s.update(xs8_t=xs8_t, xs16_t=xs16_t)

        def emit_qk(s):
            # 8 blocks: q(g,dhi) x4 then k(g,dhi) x4; q gets the fp8 bias matmul
            qdr = [qkp.tile([128, 1024], fp8, tag=f"q{g}", name=f"qdr{g}")
                   for g in range(2)]
            kdr = [qkp.tile([128, 1024], fp8, tag=f"k{g}", name=f"kdr{g}")
                   for g in range(2)]
            rhs = _ap(s["xs8_t"], 0, [[512, 2], [1, 512]])
            for m in range(8):
                is_q = m < 4
                g, dhi = (m % 4) // 2, m % 2
                ps = pE.tile([128, 512], f32, tag="pe")
                nc.tensor.matmul(ps[:],
                                 _ap(wqk8_sb, m * 256, [[128, 2], [1, 128]]),
                                 rhs, perf_mode=DR, start=True, stop=True)
                dst = (qdr if is_q else kdr)[g][:, dhi * 512:(dhi + 1) * 512]
                if is_q:
                    nc.scalar.activation(dst, ps[:], AF.Identity,
                                         bias=qb_sb[:, m:m + 1])
                else:
                    nc.scalar.copy(dst, ps[:])
            s.update(qdr=qdr, kdr=kdr)

        def emit_v(s, w):
            v8_t = v8p.tile([128, 1024], fp8, tag="v8")
            v16_t = v16p.tile([128, 1024], bf16, tag="v16")
            for ot in range(2):
                ps = pE.tile([128, 512], f32, tag="pe")
                for kt in range(2):
                    nc.tensor.matmul(ps[:],
                                     wv16_sb[:, kt * 256 + ot * 128:kt * 256 + (ot + 1) * 128],
                                     s["xs16_t"][:, kt * 512:(kt + 1) * 512],
                                     start=(kt == 0), stop=False)
                nc.tensor.matmul(ps[:], vb16_sb[:, ot * 128:(ot + 1) * 128],
                                 ones16r_sb[:], start=False, stop=True)
                cs = slice(ot * 512, (ot + 1) * 512)
                nc.vector.tensor_copy(v8_t[:, cs], ps[:])
                nc.vector.tensor_copy(v16_t[:, cs], ps[:])
            for ot2 in range(2):
                ps2 = pE.tile([128, 512], f32, tag="pe")
                for kt in range(2):
                    nc.tensor.matmul(ps2[:],
                                     wob16_sb[:, kt * 256 + ot2 * 128:kt * 256 + (ot2 + 1) * 128],
                                     v16_t[:, kt * 512:(kt + 1) * 512],
                                     start=(kt == 0), stop=(kt == 1))
                voe = v16p.tile([128, 512], f32, tag=f"voe{ot2}")
                nc.vector.tensor_copy(voe[:], ps2[:])
                nc.sync.dma_start(voutp[ot2 * 128:(ot2 + 1) * 128, s["sl"]],
                                  voe[:])
            va = va_sb[w % 2]
            for ct in range(2):
                trp = pM.tile([128, 512], bf16, tag="pm")
                for jt in range(4):
                    nc.tensor.transpose(
                        trp[:, jt * 128:(jt + 1) * 128],
                        v16_t[:, ct * 512 + jt * 128:ct * 512 + (jt + 1) * 128],
                        ident16_sb[:])
                dst = _ap(va, 256 * ct + 32, [[512, 4], [64, 4], [1, 32]])
                nc.vector.tensor_copy(
                    dst, trp[:].rearrange("p (a b c) -> p a b c", a=4, b=4))
            vpw = [vpw_sb[ct][w % 2] for ct in range(2)]
            for ct in range(2):
                dst = _ap(vpw[ct], CP + 1, [[CP, R], [1, C]])
                vsrc = v8_t[:, ct * 512:(ct + 1) * 512].rearrange(
                    "p (r c) -> p r c", r=R)
                nc.gpsimd.tensor_copy(dst, vsrc)
            s.update(v8_t=v8_t, va=va, vpw=vpw)
            otf_t = otfP.tile([128, 1024], fp8, tag="otf")
            s["otf_t"] = otf_t

        def emit_heads(s, h0, h1):
            qt, kt, va, otf_t = s["qt"], s["kt"], s["va"], s["otf_t"]
            for hp in range(h0, h1, 2):
                pair = (hp, hp + 1)
                exs = {}
                for h in pair:
                    exs[h] = exP.tile([128, 2048], fp8, tag="ex",
                                      name=f"ex{h % 2}")
                for half in range(2):
                    sts = {}
                    for h in pair:
                        sts[h] = pST.tile([128, 1024], f32, tag="st",
                                          name=f"st{h % 2}")
                    for j2 in range(2):
                        jt = half * 2 + j2
                        for h in pair:
                            cth, hh = h // 4, h % 4
                            nc.tensor.matmul(
                                sts[h][:, j2 * 512:(j2 + 1) * 512],
                                kt[32 * hh:32 * hh + 32,
                                   cth * 512 + jt * 128:cth * 512 + (jt + 1) * 128],
                                qt[32 * hh:32 * hh + 32,
                                   cth * 512:(cth + 1) * 512],
                                start=True, stop=True,
                                tile_position=(32 * hh, 0))
                    for h in pair:
                        nc.scalar.activation(
                            exs[h][:, half * 1024:(half + 1) * 1024],
                            sts[h][:], AF.Exp, scale=2.0 ** (-2 * EQ))
                for h in pair:
                    cth, hh = h // 4, h % 4
                    pv = pM.tile([64, 512], f32, tag="pm")
                    for half in range(2):
                        lhs = _ap(va, 1024 * half + 64 * h, [[512, 2], [1, 64]])
                        rhs = _ap(exs[h], 1024 * half, [[512, 2], [1, 512]])
                        nc.tensor.matmul(pv[:], lhs, rhs, perf_mode=DR,
                                         start=(half == 0), stop=(half == 1))
                    rec = recP.tile([32, 512], f32, tag="rec")
                    nc.vector.reciprocal_approx_fast(rec[:], pv[0:32, :])
                    nc.vector.tensor_mul(
                        otf_t[32 * hh:32 * hh + 32, cth * 512:(cth + 1) * 512],
                        pv[32:64, :], rec[:])

        def emit_tail(s):
            otf_t = s["otf_t"]
            for ct in range(2):
                lp = pM.tile([128, 512], f32, tag="pm", name="lp")
                for t9 in range(9):
                    dy, dx = TAPS9[t9]
                    rhs = _ap(s["vpw"][ct], CP + 1 + CP * dy + dx,
                              [[CP, R], [1, C]])
                    nc.tensor.matmul(
                        _ap(lp, 0, [[1, R * C]]),
                        ld8_sb[:, (ct * 9 + t9) * 128:(ct * 9 + t9 + 1) * 128],
                        rhs, start=(t9 == 0), stop=(t9 == 8),
                        skip_group_check=(t9 not in (0, 8)))
                cs = slice(ct * 512, (ct + 1) * 512)
                nc.vector.scalar_tensor_tensor(
                    otf_t[:, cs], lp[:], 2.0 ** (SO - SV - EL), otf_t[:, cs],
                    op0=ALU.mult, op1=ALU.add)
            for ot in range(2):
                pp = pM.tile([128, 512], f32, tag="pm")
                for kt in range(2):
                    nc.tensor.matmul(pp[:],
                                     wy16_sb[:, kt * 256 + ot * 128:kt * 256 + (ot + 1) * 128],
                                     s["xb16_t"][:, kt * 512:(kt + 1) * 512],
                                     start=(kt == 0), stop=False)
                nc.tensor.matmul(pp[:],
                                 _ap(w28_sb, ot * 256, [[128, 2], [1, 128]]),
                                 _ap(otf_t, 0, [[512, 2], [1, 512]]),
                                 perf_mode=DR, start=False, stop=True)
                outf = outfP.tile([128, 512], f32, tag="outf")
                nc.vector.tensor_scalar(outf[:], pp[:], 2.0 ** -(EW2 + SO),
                                        b2c_sb[:, ot:ot + 1],
                                        op0=ALU.mult, op1=ALU.add)
                nc.sync.dma_start(outp[ot * 128:(ot + 1) * 128, s["sl"]],
                                  outf[:])

        # prologue
        cur = emit_A1(0)
        emit_stats(cur)
        emit_xs(cur)
        emit_qk(cur)
        emit_v(cur, 0)
        for w in range(8):
            nxt = emit_A1(w + 1) if w + 1 < 8 else None
            emit_heads(cur, 0, 2)
            if nxt:
                emit_stats(nxt)
            emit_heads(cur, 2, 4)
            if nxt:
                emit_xs(nxt)
            emit_heads(cur, 4, 6)
            if nxt:
                emit_v(nxt, w + 1)
            emit_heads(cur, 6, 8)
            if nxt:
                emit_qk(nxt)
            emit_tail(cur)
            cur = nxt

    nc.compile()
    return nc


# ---------------------------------------------------------------------------
# host side
# ---------------------------------------------------------------------------

def _perms():
    p0 = np.arange(L).reshape(64, 8, 8).transpose(1, 0, 2).ravel()
    p1 = np.arange(L).reshape(8, 8, 64).transpose(0, 2, 1).ravel()
    return p0, p1


F8 = ml_dtypes.float8_e4m3fn
BF = ml_dtypes.bfloat16


def _qk_perm():
    """Channel permutation for q/k dr layout.

    Block m = (is_k, g, dhi); psum partition p: band h'=p//32, r=p%32;
    r<16 -> source channel (4g+h')*32 + dhi*16 + r, else junk (zero col).
    Returns [8, 128] channel index or -1."""
    idx = np.full((8, 128), -1, np.int64)
    for m in range(8):
        g, dhi = (m % 4) // 2, m % 2
        for p in range(128):
            hp, r = p // 32, p % 32
            if r < 16:
                idx[m, p] = (4 * g + hp) * 32 + dhi * 16 + r
    return idx


def _host_prep(inputs):
    x = np.asarray(inputs['x'], np.float32)
    w_embed = np.asarray(inputs['w_embed'], np.float32)
    g1 = np.asarray(inputs['g1'], np.float32)
    b1 = np.asarray(inputs['b1'], np.float32)
    w_qkv = np.asarray(inputs['w_qkv'], np.float32)
    w_proj = np.asarray(inputs['w_proj'], np.float32)
    b_proj = np.asarray(inputs['b_proj'], np.float32)
    w_out = np.asarray(inputs['w_out'], np.float32)
    conv_w = [np.asarray(inputs['conv_w0'], np.float32),
              np.asarray(inputs['conv_w1'], np.float32)]
    conv_b = [np.asarray(inputs['conv_b0'], np.float32),
              np.asarray(inputs['conv_b1'], np.float32)]
    perms = _perms()
    qkidx = _qk_perm()

    mvec = w_embed.mean(axis=1)
    G = w_embed @ w_embed.T
    Wm = w_embed - mvec[:, None]
    ident = np.eye(128, dtype=np.float32).astype(BF)

    def pack_kt_ot(M):
        # [256 in, 256 out] -> [128, 512]: [k_lo, kt*256 + m]
        out = np.zeros((128, 512), np.float32)
        for kt in range(2):
            out[:, kt * 256:(kt + 1) * 256] = M[kt * 128:(kt + 1) * 128, :]
        return out

    in_maps = []
    for c in range(8):
        b, br = c // 2, c % 2
        pm = perms[br]
        o = br * 256
        xp = np.ascontiguousarray(x[b].reshape(256, L)[:, pm])
        xb8 = np.concatenate([xp[:128], xp[128:]], axis=1)  # [128, 2L]
        gWq = g1[:, None] * w_qkv[:, o:o + 256] * SCALE
        gWk = g1[:, None] * w_qkv[:, 512 + o:512 + o + 256]
        gWv = g1[:, None] * w_qkv[:, 1024 + o:1024 + o + 256]
        Wq_ = Wm @ gWq
        Wk_ = Wm @ gWk
        Wv_ = Wm @ gWv
        bq = (b1 @ w_qkv[:, o:o + 256]) * SCALE
        bv = b1 @ w_qkv[:, 1024 + o:1024 + o + 256]
        woutb = w_out[o:o + 256]
        wy = w_embed[:, o:o + 256] @ woutb
        w2 = w_proj[o:o + 256] @ w_out
        b2 = ((b_proj @ w_out) if br == 0 else np.zeros(256, np.float32)) \
            + conv_b[br] @ w2
        w9 = conv_w[br].reshape(256, 3, 3)
        if br == 1:
            w9 = w9.transpose(0, 2, 1)
        w9 = np.ascontiguousarray(w9.reshape(256, 9))

        # wqk8: 8 blocks of [128, 2 planes x 128 cols]
        wqk8 = np.zeros((128, 2048), np.float32)
        qb = np.zeros((128, 4), np.float32)
        for m in range(8):
            W = Wq_ if m < 4 else Wk_
            for kt in range(2):
                blk = wqk8[:, m * 256 + kt * 128:m * 256 + (kt + 1) * 128]
                for p in range(128):
                    ch = qkidx[m, p]
                    if ch >= 0:
                        blk[:, p] = W[kt * 128:(kt + 1) * 128, ch] * 2.0 ** EQ
            if m < 4:  # q bias column for the ACT Identity copy
                for p in range(128):
                    ch = qkidx[m, p]
                    if ch >= 0:
                        qb[p, m] = bq[ch] * 2.0 ** EQ

        # ldiag8: (ct, pair) blocks of [128, 2 planes x 128], diag values
        ld8 = np.zeros((128, 2560), np.float32)
        for ct in range(2):
            for t9 in range(9):
                blk = ld8[:, (ct * 9 + t9) * 128:(ct * 9 + t9 + 1) * 128]
                blk[np.arange(128), np.arange(128)] = \
                    w9[ct * 128:(ct + 1) * 128, t9] * 2.0 ** EL

        w28 = np.zeros((128, 512), np.float32)
        for ot in range(2):
            for kt in range(2):
                w28[:, ot * 256 + kt * 128:ot * 256 + (kt + 1) * 128] = \
                    w2[kt * 128:(kt + 1) * 128,
                       ot * 128:(ot + 1) * 128] * 2.0 ** EW2

        in_maps.append({
            "xb16": xb8.astype(BF),
            "g16": pack_kt_ot(G).astype(BF),
            "mv16": mvec.reshape(2, 128).T.astype(BF),
            "wy16": (pack_kt_ot(wy) * 2.0 ** (EW2 + SO)).astype(BF),
            "wv16": (pack_kt_ot(Wv_) * 2.0 ** SV).astype(BF),
            "wob16": (pack_kt_ot(woutb) * 2.0 ** -SV).astype(BF),
            "wqk8": wqk8.astype(F8),
            "qb": qb,
            "vb16": (bv.reshape(1, 256) * 2.0 ** SV).astype(BF),
            "ld8": ld8.astype(F8),
            "w28": w28.astype(F8),
            "b2c": np.ascontiguousarray(b2.reshape(2, 128).T.astype(np.float32)),
            "ident16": ident,
        })
    return in_maps


def _gather(results):
    perms = _perms()
    out = np.zeros((B, 256, L), np.float32)
    vout = np.zeros((B, 256, L), np.float32)
    for c in range(8):
        b, br = c // 2, c % 2
        pm = perms[br]
        tmp = np.zeros((256, L), np.float32)
        tmp[:, pm] = results[c]["outp"]
        out[b] += tmp
        tmp = np.zeros((256, L), np.float32)
        tmp[:, pm] = results[c]["voutp"]
        vout[b] += tmp
    return (out.reshape(B, 256, 64, 64), vout.reshape(B, 256, 64, 64))


_CACHE = {}


def get_nc():
    if "nc" not in _CACHE:
        _CACHE["nc"] = build_nc()
    return _CACHE["nc"]


def kernel(**inputs):
    nc = get_nc()
    in_maps = _host_prep(inputs)
    res = run_bass_kernel_spmd(nc, in_maps, core_ids=list(range(8)))
    return _gather(res.results)
